# revision 1
# baseline (speedup 1.0000x reference)
"""GATv2 (2-layer) + link-prediction scores on 8 Trainium2 NeuronCores.

Strategy (dst-block sharding):
- Sort edges by dst on host; shard 128-dst blocks across 8 cores (49 slots/core).
- Node projections (fs/fd tables, bf16) computed replicated on every core.
- Edge pass per 128-edge chunk: indirect-gather F[src] rows only; G rows for the
  whole dst block are gathered ONCE per block and broadcast per-edge on the PE:
  t = S_T @ G_blk + I @ F in PSUM fp32 (S_T = PE-transpose of the one-hot S).
  Then Prelu(0.2); attn-weighted reduce -> logits; exp -> ex; wmsg = ex*t;
  one-hot S matmul accumulates [sum wmsg | sum ex] per dst block in PSUM.
  out[v] = (sum ex*t)/denom - G[v]  (since sum alpha = 1).
- Layer-1 output stored transposed (h1T) for layer-2 projections; AllGather
  between layers; layer-2 output row-major, AllGather, then query dot products.
"""
import sys
import numpy as np

sys.path.insert(0, '/opt/trn_rl_repo')

import ml_dtypes

N_NODES = 50000
N_EDGES = 800000
N_QUERY = 100000
HD = 128          # heads*dim
H, D = 4, 32
NEG = 0.2
CORES = 8
P = 128
BPC = 49                       # dst blocks per core
NB = CORES * BPC               # 392 block slots
NPAD = NB * P                  # 50176 padded nodes
SH = BPC * P                   # 6272 nodes per core shard
QPC = 98                       # query chunks per core (98*128 = 12544 >= 12500)
QREAL = N_QUERY // CORES       # 12500
EDGE_GROUP = 4                 # chunks per group (= 1 psum bank of t)

bf16 = ml_dtypes.bfloat16

_program_cache = {}


def _preprocess(inputs):
    feat = np.asarray(inputs['feat'], np.float32)
    es = np.asarray(inputs['edge_src']).astype(np.int64)
    ed = np.asarray(inputs['edge_dst']).astype(np.int64)
    qs = np.asarray(inputs['qsrc']).astype(np.int64)
    qd = np.asarray(inputs['qdst']).astype(np.int64)

    order = np.argsort(ed, kind='stable')
    es_s = es[order].astype(np.int32)
    ed_s = ed[order].astype(np.int32)
    blk = (ed_s >> 7).astype(np.int64)            # global block id of each edge
    counts = np.bincount(blk, minlength=NB)       # [NB]
    starts = np.zeros(NB + 1, np.int64)
    np.cumsum(counts, out=starts[1:])
    cnt_cs = counts.reshape(CORES, BPC)
    K = np.maximum(((cnt_cs + P - 1) // P).max(axis=0), 1).astype(np.int64)  # [BPC]
    chunk_base = np.zeros(BPC + 1, np.int64)
    np.cumsum(K, out=chunk_base[1:])
    TC = int(chunk_base[-1])                      # chunks per core

    E = len(es_s)
    gsrc = np.zeros((CORES, TC * P), np.int32)
    gdst = np.zeros((CORES, TC * P), np.int32)
    dstl = np.full((CORES, TC * P), float(P), np.float32)
    core_of = blk // BPC
    slot_of = blk % BPC
    pos = chunk_base[slot_of] * P + (np.arange(E) - starts[blk])
    gsrc[core_of, pos] = es_s
    gdst[core_of, pos] = ed_s
    dstl[core_of, pos] = (ed_s - blk * P).astype(np.float32)

    # lane-major layouts [P, TC] per core
    gsrcT = gsrc.reshape(CORES, TC, P).transpose(0, 2, 1).copy()
    gdstT = gdst.reshape(CORES, TC, P).transpose(0, 2, 1).copy()
    dstlT = dstl.reshape(CORES, TC, P).transpose(0, 2, 1).astype(bf16).copy()

    # queries: core c handles [c*QREAL, (c+1)*QREAL), padded to QPC*P with 0
    qsT = np.zeros((CORES, QPC * P), np.int32)
    qdT = np.zeros((CORES, QPC * P), np.int32)
    for c in range(CORES):
        qsT[c, :QREAL] = qs[c * QREAL:(c + 1) * QREAL]
        qdT[c, :QREAL] = qd[c * QREAL:(c + 1) * QREAL]
    qsT = qsT.reshape(CORES, QPC, P).transpose(0, 2, 1).copy()
    qdT = qdT.reshape(CORES, QPC, P).transpose(0, 2, 1).copy()

    featT = np.zeros((P, NPAD), np.float32)
    featT[:, :N_NODES] = feat.T
    featT = featT.astype(bf16)

    def wcat(a, b):
        return np.concatenate([np.asarray(a, np.float32),
                               np.asarray(b, np.float32)], axis=1).astype(bf16)

    WB1 = wcat(inputs['W1_src'], inputs['W1_dst'])        # [128, 256]
    WB2 = wcat(inputs['W2_src'], inputs['W2_dst'])
    bias1 = np.concatenate([np.asarray(inputs['b1_src'], np.float32),
                            np.asarray(inputs['b1_dst'], np.float32)])
    bias2 = np.concatenate([np.asarray(inputs['b2_src'], np.float32),
                            np.asarray(inputs['b2_dst'], np.float32)])
    has_bias = bool(np.any(bias1) or np.any(bias2))
    A1 = np.tile(np.asarray(inputs['attn1'], np.float32).reshape(1, HD), (P, 1)).astype(bf16)
    A2 = np.tile(np.asarray(inputs['attn2'], np.float32).reshape(1, HD), (P, 1)).astype(bf16)
    IOTA = np.tile(np.arange(P, dtype=np.float32), (P, 1)).astype(bf16)

    blkidT = np.zeros((CORES, P, BPC), np.int32)
    for c in range(CORES):
        ids = c * SH + np.arange(SH, dtype=np.int32)
        blkidT[c] = ids.reshape(BPC, P).T

    common = {
        'featT': featT, 'WB1': WB1, 'WB2': WB2,
        'A1': A1, 'A2': A2, 'IOTA': IOTA,
        'B1': bias1.reshape(1, 2 * HD).astype(bf16),
        'B2': bias2.reshape(1, 2 * HD).astype(bf16),
    }
    in_maps = []
    for c in range(CORES):
        m = dict(common)
        m['gsrcT'] = gsrcT[c]
        m['gdstT'] = gdstT[c]
        m['dstlT'] = dstlT[c]
        m['qsT'] = qsT[c]
        m['qdT'] = qdT[c]
        m['blkidT'] = blkidT[c]
        in_maps.append(m)
    return in_maps, tuple(K.tolist()), has_bias


def _split_multiwait(nc, max_waits=1):
    """walrus in this env rejects >1 sync wait per instruction; split extras
    into per-wait NoOps inserted before."""
    import concourse.mybir as mybir
    f = nc.m.functions[0]
    for bb in f.blocks:
        insts = list(bb.instructions)
        out = []
        changed = False
        for inst in insts:
            si = inst.sync_info
            if si is not None and len(si.on_wait) > max_waits:
                waits = list(si.on_wait)
                head, tail = waits[:-max_waits], waits[-max_waits:]
                for i, w in enumerate(head):
                    out.append(mybir.InstNoOp(
                        name=f"{inst.name}_sw{i}",
                        sync_info=mybir.SyncInfo(on_wait=[w], on_update=[]),
                        bass_nofuse=True,
                        engine=inst.engine,
                    ))
                inst.sync_info = mybir.SyncInfo(on_wait=tail,
                                                on_update=list(si.on_update))
                changed = True
            out.append(inst)
        if changed:
            bb.instructions = out
    return nc


def _build_program(K, has_bias):
    import concourse.bass as bass
    import concourse.mybir as mybir
    import concourse.tile as tile
    from concourse.masks import make_identity

    K = list(K)
    TC = sum(K)
    nc = bass.Bass(num_swdge_queues=4)
    dyn_units = []  # lists of indirect-DMA insts sharing one queue
    dt = mybir.dt

    featT_d = nc.declare_dram_parameter("featT", [P, NPAD], dt.bfloat16, isOutput=False)
    WB1_d = nc.declare_dram_parameter("WB1", [P, 2 * HD], dt.bfloat16, isOutput=False)
    WB2_d = nc.declare_dram_parameter("WB2", [P, 2 * HD], dt.bfloat16, isOutput=False)
    A1_d = nc.declare_dram_parameter("A1", [P, HD], dt.bfloat16, isOutput=False)
    A2_d = nc.declare_dram_parameter("A2", [P, HD], dt.bfloat16, isOutput=False)
    IOTA_d = nc.declare_dram_parameter("IOTA", [P, P], dt.bfloat16, isOutput=False)
    B1_d = nc.declare_dram_parameter("B1", [1, 2 * HD], dt.bfloat16, isOutput=False)
    B2_d = nc.declare_dram_parameter("B2", [1, 2 * HD], dt.bfloat16, isOutput=False)
    gsrc_d = nc.declare_dram_parameter("gsrcT", [P, TC], dt.int32, isOutput=False)
    gdst_d = nc.declare_dram_parameter("gdstT", [P, TC], dt.int32, isOutput=False)
    dstl_d = nc.declare_dram_parameter("dstlT", [P, TC], dt.bfloat16, isOutput=False)
    qs_d = nc.declare_dram_parameter("qsT", [P, QPC], dt.int32, isOutput=False)
    qd_d = nc.declare_dram_parameter("qdT", [P, QPC], dt.int32, isOutput=False)
    blkid_d = nc.declare_dram_parameter("blkidT", [P, BPC], dt.int32, isOutput=False)
    scores_d = nc.declare_dram_parameter("scores", [P, QPC], dt.float32, isOutput=True)

    with tile.TileContext(nc) as tc:
        with (
            tc.tile_pool(name="const", bufs=1) as cpool,
            tc.tile_pool(name="proj", bufs=4) as ppool,
            tc.tile_pool(name="edge", bufs=3) as epool,
            tc.tile_pool(name="fin", bufs=2) as fpool,
            tc.tile_pool(name="ppsum", bufs=1, space="PSUM") as ppsum,
            tc.tile_pool(name="apsum", bufs=2, space="PSUM") as apsum,
            tc.tile_pool(name="tpsum", bufs=2, space="PSUM") as tpsum,
            tc.tile_pool(name="t2psum", bufs=2, space="PSUM") as t2psum,
            tc.tile_pool(name="dram", bufs=1, space="DRAM") as dpool,
        ):
            # ---- constants resident in SBUF ----
            ident = cpool.tile([P, P], dt.bfloat16)
            make_identity(nc, ident[:])
            wb1_t = cpool.tile([P, 2 * HD], dt.bfloat16)
            nc.sync.dma_start(out=wb1_t[:], in_=WB1_d[:])
            wb2_t = cpool.tile([P, 2 * HD], dt.bfloat16)
            nc.sync.dma_start(out=wb2_t[:], in_=WB2_d[:])
            a1_t = cpool.tile([P, HD], dt.bfloat16)
            nc.sync.dma_start(out=a1_t[:], in_=A1_d[:])
            a2_t = cpool.tile([P, HD], dt.bfloat16)
            nc.sync.dma_start(out=a2_t[:], in_=A2_d[:])
            iota_t = cpool.tile([P, P], dt.bfloat16)
            nc.sync.dma_start(out=iota_t[:], in_=IOTA_d[:])
            gsrc_t = cpool.tile([P, TC], dt.int32)
            nc.sync.dma_start(out=gsrc_t[:], in_=gsrc_d[:])
            gdst_t = cpool.tile([P, TC], dt.int32)
            nc.sync.dma_start(out=gdst_t[:], in_=gdst_d[:])
            dstl_t = cpool.tile([P, TC], dt.bfloat16)
            nc.sync.dma_start(out=dstl_t[:], in_=dstl_d[:])
            qs_t = cpool.tile([P, QPC], dt.int32)
            nc.sync.dma_start(out=qs_t[:], in_=qs_d[:])
            qd_t = cpool.tile([P, QPC], dt.int32)
            nc.sync.dma_start(out=qd_t[:], in_=qd_d[:])
            blkid_t = cpool.tile([P, BPC], dt.int32)
            nc.sync.dma_start(out=blkid_t[:], in_=blkid_d[:])
            if has_bias:
                ones_t = cpool.tile([1, 2 * HD], dt.bfloat16)
                nc.gpsimd.memset(ones_t[:], 1.0)
                b1_t = cpool.tile([1, 2 * HD], dt.bfloat16)
                nc.sync.dma_start(out=b1_t[:], in_=B1_d[:])
                b2_t = cpool.tile([1, 2 * HD], dt.bfloat16)
                nc.sync.dma_start(out=b2_t[:], in_=B2_d[:])

            # ---- internal DRAM ----
            F1_d = dpool.tile([NPAD, HD], dt.bfloat16)
            G1_d = dpool.tile([NPAD, HD], dt.bfloat16)
            F2_d = dpool.tile([NPAD, HD], dt.bfloat16)
            G2_d = dpool.tile([NPAD, HD], dt.bfloat16)
            h1T_sh = dpool.tile([P, SH], dt.bfloat16)
            h1T_ag = dpool.tile([CORES, P, SH], dt.bfloat16, addr_space="Shared")
            h2_sh = dpool.tile([SH, HD], dt.bfloat16)
            h2_ag = dpool.tile([NPAD, HD], dt.bfloat16, addr_space="Shared")

            def projections(groups, w_t, b_t, F_out, G_out):
                """F/G tables. groups: list of (lhsT_ap [P, g*P], nt0, g)."""
                for lhsT_ap, nt0, g in groups:
                    lhsT = ppool.tile([P, 4 * P], dt.bfloat16, tag="lhsT")
                    nc.sync.dma_start(out=lhsT[:, 0:g * P], in_=lhsT_ap)
                    fgsb = ppool.tile([P, 4, 2 * HD], dt.bfloat16, tag="fgsb")
                    for h in range(0, g, 2):    # psum banks of up to 2 tiles
                        gh = min(2, g - h)
                        fp = ppsum.tile([P, 2, 2 * HD], dt.float32, space="PSUM",
                                        tag="fp")
                        for j in range(gh):
                            sl = lhsT[:, (h + j) * P:(h + j + 1) * P]
                            if has_bias:
                                nc.tensor.matmul(out=fp[:, j, :], lhsT=ones_t[:],
                                                 rhs=b_t[:], start=True, stop=False)
                                nc.tensor.matmul(out=fp[:, j, :], lhsT=sl,
                                                 rhs=w_t[:], start=False, stop=True)
                            else:
                                nc.tensor.matmul(out=fp[:, j, :], lhsT=sl,
                                                 rhs=w_t[:], start=True, stop=True)
                        nc.scalar.copy(out=fgsb[:, h:h + gh, :], in_=fp[:, 0:gh, :])
                    nc.sync.dma_start(
                        out=F_out[nt0 * P:(nt0 + g) * P, :]
                            .rearrange("(c p) d -> p c d", p=P),
                        in_=fgsb[:, 0:g, 0:HD])
                    nc.scalar.dma_start(
                        out=G_out[nt0 * P:(nt0 + g) * P, :]
                            .rearrange("(c p) d -> p c d", p=P),
                        in_=fgsb[:, 0:g, HD:2 * HD])

            def edge_pass(F_tab, G_tab, a_t, layer):
                base = 0
                for i in range(BPC):
                    Ki = K[i]
                    g_blk = fpool.tile([P, HD], dt.bfloat16, tag="gblk")
                    iB = nc.gpsimd.indirect_dma_start(
                        out=g_blk[:], out_offset=None, in_=G_tab[:],
                        in_offset=bass.IndirectOffsetOnAxis(
                            ap=blkid_t[:, i:i + 1], axis=0),
                    )
                    dyn_units.append([iB.ins])
                    acc_p = apsum.tile([P, HD + 4], dt.float32, space="PSUM", tag="agg")
                    for g0 in range(0, Ki, EDGE_GROUP):
                        gs = min(EDGE_GROUP, Ki - g0)
                        F_t = epool.tile([P, EDGE_GROUP, HD], dt.bfloat16, tag="F")
                        for j in range(gs):
                            tch = base + g0 + j
                            iF = nc.gpsimd.indirect_dma_start(
                                out=F_t[:, j, :], out_offset=None,
                                in_=F_tab[:],
                                in_offset=bass.IndirectOffsetOnAxis(
                                    ap=gsrc_t[:, tch:tch + 1], axis=0),
                            )
                            dyn_units.append([iF.ins])
                        S_t = epool.tile([P, EDGE_GROUP, P], dt.bfloat16, tag="S")
                        nc.vector.tensor_tensor(
                            out=S_t[:, 0:gs, :],
                            in0=dstl_t[:, base + g0:base + g0 + gs]
                                .unsqueeze(2).to_broadcast([P, gs, P]),
                            in1=iota_t[:].unsqueeze(1).to_broadcast([P, gs, P]),
                            op=mybir.AluOpType.is_equal,
                        )
                        st_p = tpsum.tile([P, EDGE_GROUP, P], dt.bfloat16,
                                          space="PSUM", tag="tp")
                        for j in range(gs):
                            nc.tensor.transpose(out=st_p[:, j, :], in_=S_t[:, j, :],
                                                identity=ident[:])
                        sT_t = epool.tile([P, EDGE_GROUP, P], dt.bfloat16, tag="sT")
                        nc.scalar.copy(out=sT_t[:, 0:gs, :], in_=st_p[:, 0:gs, :])
                        t_p = t2psum.tile([P, EDGE_GROUP, HD], dt.float32,
                                          space="PSUM", tag="t2")
                        for j in range(gs):
                            nc.tensor.matmul(out=t_p[:, j, :], lhsT=sT_t[:, j, :],
                                             rhs=g_blk[:], start=True, stop=False)
                            nc.tensor.matmul(out=t_p[:, j, :], lhsT=ident[:],
                                             rhs=F_t[:, j, :], start=False, stop=True)
                        L_t = epool.tile([P, EDGE_GROUP, HD], dt.bfloat16, tag="L")
                        nc.scalar.activation(
                            out=L_t[:, 0:gs, :], in_=t_p[:, 0:gs, :],
                            func=mybir.ActivationFunctionType.Prelu, alpha=NEG,
                        )
                        nc.vector.tensor_tensor(
                            out=L_t[:, 0:gs, :], in0=L_t[:, 0:gs, :],
                            in1=a_t[:].unsqueeze(1).to_broadcast([P, gs, HD]),
                            op=mybir.AluOpType.mult,
                        )
                        lg_t = epool.tile([P, EDGE_GROUP, H], dt.float32, tag="lg")
                        nc.vector.tensor_reduce(
                            out=lg_t[:, 0:gs, :],
                            in_=L_t[:, 0:gs, :].rearrange("p c (h d) -> p c h d", d=D),
                            axis=mybir.AxisListType.X, op=mybir.AluOpType.add,
                        )
                        wm_t = epool.tile([P, EDGE_GROUP, HD + 4], dt.bfloat16, tag="wm")
                        nc.scalar.activation(
                            out=wm_t[:, 0:gs, HD:HD + 4], in_=lg_t[:, 0:gs, :],
                            func=mybir.ActivationFunctionType.Exp,
                        )
                        nc.vector.tensor_tensor(
                            out=wm_t[:, 0:gs, 0:HD].rearrange("p c (h d) -> p c h d", d=D),
                            in0=t_p[:, 0:gs, :].rearrange("p c (h d) -> p c h d", d=D),
                            in1=wm_t[:, 0:gs, HD:HD + 4]
                                .unsqueeze(3).to_broadcast([P, gs, H, D]),
                            op=mybir.AluOpType.mult,
                        )
                        for j in range(gs):
                            nc.tensor.matmul(
                                out=acc_p[:], lhsT=S_t[:, j, :], rhs=wm_t[:, j, :],
                                start=(g0 + j == 0), stop=(g0 + j == Ki - 1),
                            )
                    base += Ki
                    # ---- finalize block i ----
                    den_t = fpool.tile([P, H], dt.float32, tag="den")
                    nc.vector.tensor_scalar_add(out=den_t[:], in0=acc_p[:, HD:HD + 4],
                                                scalar1=1e-30)
                    rden_t = fpool.tile([P, H], dt.float32, tag="rden")
                    nc.vector.reciprocal(out=rden_t[:], in_=den_t[:])
                    o_t = fpool.tile([P, HD], dt.bfloat16, tag="ot")
                    nc.vector.tensor_tensor(
                        out=o_t[:].rearrange("p (h d) -> p h d", d=D),
                        in0=acc_p[:, 0:HD].rearrange("p (h d) -> p h d", d=D),
                        in1=rden_t[:].unsqueeze(2).to_broadcast([P, H, D]),
                        op=mybir.AluOpType.mult,
                    )
                    if layer == 1:
                        nc.vector.tensor_tensor(out=o_t[:], in0=o_t[:], in1=g_blk[:],
                                                op=mybir.AluOpType.subtract)
                        r_t = fpool.tile([P, HD], dt.bfloat16, tag="rt")
                        nc.scalar.activation(out=r_t[:], in_=o_t[:],
                                             func=mybir.ActivationFunctionType.Relu)
                        tp = tpsum.tile([P, P], dt.bfloat16, space="PSUM", tag="tp")
                        nc.tensor.transpose(out=tp[:], in_=r_t[:], identity=ident[:])
                        tsb = fpool.tile([P, P], dt.bfloat16, tag="tsb")
                        nc.scalar.copy(out=tsb[:], in_=tp[:])
                        nc.sync.dma_start(out=h1T_sh[:, i * P:(i + 1) * P], in_=tsb[:])
                    else:
                        nc.vector.tensor_tensor(out=o_t[:], in0=o_t[:], in1=g_blk[:],
                                                op=mybir.AluOpType.subtract)
                        nc.sync.dma_start(out=h2_sh[i * P:(i + 1) * P, :], in_=o_t[:])

            # ---- phase 1: layer-1 projections ----
            skip_proj = bool(globals().get('SKIP_PROJ', False))
            p1_groups = [
                (featT_d[:, st * 4 * P:(st * 4 + 4) * P], st * 4, 4)
                for st in range(NB // 4)
            ]
            p4_groups = []
            for cn in range(CORES):
                for st in range(0, BPC, 4):
                    g = min(4, BPC - st)
                    p4_groups.append(
                        (h1T_ag[cn, :, st * P:(st + g) * P], cn * BPC + st, g))
            if not skip_proj:
                projections(p1_groups, wb1_t, b1_t if has_bias else None,
                            F1_d, G1_d)
            # ---- phase 2: layer-1 edge pass ----
            edge_pass(F1_d, G1_d, a1_t, layer=1)
            # ---- phase 3: AllGather h1T ----
            nc.gpsimd.collective_compute(
                "AllGather", mybir.AluOpType.bypass,
                replica_groups=[list(range(CORES))],
                ins=[h1T_sh[:]], outs=[h1T_ag[:]],
            )
            # ---- phase 4: layer-2 projections ----
            if not skip_proj:
                projections(p4_groups, wb2_t, b2_t if has_bias else None,
                            F2_d, G2_d)
            # ---- phase 5: layer-2 edge pass ----
            edge_pass(F2_d, G2_d, a2_t, layer=2)
            # ---- phase 6: AllGather h2 ----
            nc.gpsimd.collective_compute(
                "AllGather", mybir.AluOpType.bypass,
                replica_groups=[list(range(CORES))],
                ins=[h2_sh[:]], outs=[h2_ag[:]],
            )
            # ---- phase 7: queries ----
            QG = 8
            for q0 in range(0, QPC, QG):
                qg = min(QG, QPC - q0)
                qa_t = epool.tile([P, QG, HD], dt.bfloat16, tag="qa")
                qb_t = epool.tile([P, QG, HD], dt.bfloat16, tag="qb")
                for j in range(qg):
                    iA = nc.gpsimd.indirect_dma_start(
                        out=qa_t[:, j, :], out_offset=None, in_=h2_ag[:],
                        in_offset=bass.IndirectOffsetOnAxis(
                            ap=qs_t[:, q0 + j:q0 + j + 1], axis=0),
                    )
                    iB2 = nc.gpsimd.indirect_dma_start(
                        out=qb_t[:, j, :], out_offset=None, in_=h2_ag[:],
                        in_offset=bass.IndirectOffsetOnAxis(
                            ap=qd_t[:, q0 + j:q0 + j + 1], axis=0),
                    )
                    dyn_units.append([iA.ins])
                    dyn_units.append([iB2.ins])
                nc.vector.tensor_tensor(out=qa_t[:, 0:qg, :], in0=qa_t[:, 0:qg, :],
                                        in1=qb_t[:, 0:qg, :],
                                        op=mybir.AluOpType.mult)
                sc_t = fpool.tile([P, QG], dt.float32, tag="sc")
                nc.vector.tensor_reduce(out=sc_t[:, 0:qg], in_=qa_t[:, 0:qg, :],
                                        axis=mybir.AxisListType.X,
                                        op=mybir.AluOpType.add)
                sg_t = fpool.tile([P, QG], dt.float32, tag="sg")
                nc.scalar.activation(out=sg_t[:, 0:qg], in_=sc_t[:, 0:qg],
                                     func=mybir.ActivationFunctionType.Sigmoid)
                nc.sync.dma_start(out=scores_d[:, q0:q0 + qg], in_=sg_t[:, 0:qg])

    qnames = ["qPoolDynamic", "qPoolDynamic1", "qPoolDynamic2", "qPoolDynamic3"]
    for u, unit in enumerate(dyn_units):
        for inst in unit:
            inst.queue = qnames[u % 4]
    _split_multiwait(nc)
    return nc


def _get_program(K, has_bias):
    key = (K, has_bias)
    if key not in _program_cache:
        _program_cache[key] = _build_program(K, has_bias)
    return _program_cache[key]


LAST_EXEC_S = None     # wall seconds of the pure device execution (staged)

_COMMON = ('featT', 'WB1', 'WB2', 'A1', 'A2', 'IOTA', 'B1', 'B2')

_NEFF_CACHE_DIR = '/root/.cache/bass_neff'


def _install_neff_cache():
    import hashlib, os, shutil
    import concourse.bass2jax as b2j
    if getattr(b2j, '_neff_cache_installed', False):
        return
    orig = b2j.compile_bir_kernel

    def cached(bir, dirpath, neff_name="file.neff"):
        data = bir if isinstance(bir, bytes) else bir.encode()
        h = hashlib.sha256(data).hexdigest()
        os.makedirs(_NEFF_CACHE_DIR, exist_ok=True)
        cpath = os.path.join(_NEFF_CACHE_DIR, h + '.neff')
        out = os.path.join(dirpath, neff_name)
        if os.path.exists(cpath):
            shutil.copyfile(cpath, out)
            return out
        p = orig(bir, dirpath, neff_name)
        try:
            shutil.copyfile(p, cpath)
        except OSError:
            pass
        return p

    b2j.compile_bir_kernel = cached
    b2j._neff_cache_installed = True


def _get_runner(K, has_bias):
    key = ('runner', K, has_bias)
    if key in _program_cache:
        return _program_cache[key]
    import jax
    import jax.core
    from jax.sharding import Mesh, PartitionSpec, NamedSharding
    from jax.experimental.shard_map import shard_map
    import concourse.bass2jax as b2j
    import concourse.mybir as mybir

    _install_neff_cache()
    nc = _get_program(K, has_bias)
    b2j.install_neuronx_cc_hook()
    partition_name = nc.partition_id_tensor.name if nc.partition_id_tensor else None
    in_names, out_names, out_avals, out_shapes = [], [], [], []
    for alloc in nc.m.functions[0].allocations:
        if not isinstance(alloc, mybir.MemoryLocationSet):
            continue
        name = alloc.memorylocations[0].name
        if alloc.kind == "ExternalInput":
            if name != partition_name:
                in_names.append(name)
        elif alloc.kind == "ExternalOutput":
            out_names.append(name)
            shape = tuple(alloc.tensor_shape)
            npdt = mybir.dt.np(alloc.dtype)
            out_avals.append(jax.core.ShapedArray(shape, npdt))
            out_shapes.append((shape, npdt))
    n_params = len(in_names)
    all_names = list(in_names) + list(out_names)
    if partition_name is not None:
        all_names.append(partition_name)

    def _body(*args):
        operands = list(args)
        if partition_name is not None:
            operands.append(b2j.partition_id_tensor())
        return tuple(b2j._bass_exec_p.bind(
            *operands, out_avals=tuple(out_avals), in_names=tuple(all_names),
            out_names=tuple(out_names), lowering_input_output_aliases=(),
            sim_require_finite=True, sim_require_nnan=True, nc=nc))

    devices = jax.devices()[:CORES]
    mesh = Mesh(np.asarray(devices), ("core",))
    shard = PartitionSpec("core")
    repl = PartitionSpec()
    in_specs = tuple(repl if nm in _COMMON else shard for nm in in_names) \
        + (shard,) * len(out_names)
    out_specs = (shard,) * len(out_names)
    donate = tuple(range(n_params, n_params + len(out_names)))
    sharded = jax.jit(
        shard_map(_body, mesh=mesh, in_specs=in_specs, out_specs=out_specs,
                  check_rep=False),
        donate_argnums=donate, keep_unused=True)
    runner = dict(sharded=sharded, mesh=mesh, in_names=in_names,
                  out_names=out_names, out_shapes=out_shapes,
                  shard=NamedSharding(mesh, shard),
                  repl=NamedSharding(mesh, repl))
    _program_cache[key] = runner
    return runner


def kernel(**inputs):
    global LAST_EXEC_S
    import time as _time
    import jax

    in_maps, K, has_bias = _preprocess(inputs)
    r = _get_runner(K, has_bias)
    args = []
    for nm in r['in_names']:
        if nm in _COMMON:
            args.append(jax.device_put(in_maps[0][nm], r['repl']))
        else:
            cat = np.concatenate([in_maps[c][nm] for c in range(CORES)], axis=0)
            args.append(jax.device_put(cat, r['shard']))
    for shape, npdt in r['out_shapes']:
        z = np.zeros((CORES * shape[0], *shape[1:]), npdt)
        args.append(jax.device_put(z, r['shard']))
    for a in args:
        a.block_until_ready()
    t0 = _time.time()
    outs = r['sharded'](*args)
    for o in outs:
        o.block_until_ready()
    LAST_EXEC_S = _time.time() - t0
    sc = np.asarray(outs[r['out_names'].index('scores')])   # [8*P, QPC]
    sc = sc.reshape(CORES, P, QPC)
    out = np.empty(N_QUERY, np.float32)
    for c in range(CORES):
        out[c * QREAL:(c + 1) * QREAL] = sc[c].T.reshape(-1)[:QREAL]
    return out

